# revision 3
# baseline (speedup 1.0000x reference)
"""Trainium2 Bass kernel for nn_EquivariantBiLinear.

Math (per batch row b):
    pieces:  Y[k, b] = sum_nu W_g[mu, nu] * x[b, bid_g[nu*r+rho]]   (k = off_g + mu*r + rho)
    out[b, o] = 0.1 * sum_i Y[W_invperm[o*256+i], b] * x[b, i]

Strategy: data-parallel over batch on 8 cores (weights replicated).
Phase 1 computes Y with 4 group GEMMs (gathered rhs via indirect DMA);
phase 2 applies the permutation with per-o indirect row gathers from Y,
multiplies by 0.1*xT and reduces over partitions with a ones-column
matmul, accumulating 128 outputs per PSUM tile.
"""

import sys

if "/opt/trn_rl_repo" not in sys.path:
    sys.path.insert(0, "/opt/trn_rl_repo")

from contextlib import ExitStack

import numpy as np

import concourse.bacc as bacc
import concourse.bass as bass
import concourse.mybir as mybir
import concourse.tile as tile
from concourse.bass import IndirectOffsetOnAxis
from concourse.bass_utils import run_bass_kernel_spmd
from concourse.masks import make_identity

GROUPS = [(512, 1, 16384), (256, 4, 4096), (128, 16, 1024), (64, 64, 256)]
OFF = [0, 16384, 32768, 49152]
X = 256
B = 2048
NCORES = 8
BS = B // NCORES  # 256 batch rows per core

F32 = mybir.dt.float32
I32 = mybir.dt.int32


def _host_prep(W0, W1, W2, W3, bid0, bid1, bid2, bid3, W_invperm):
    """Pure layout transforms of weights/indices (no arithmetic on data)."""
    Ws = [np.asarray(W) for W in (W0, W1, W2, W3)]
    bids = [np.asarray(b).astype(np.int64) for b in (bid0, bid1, bid2, bid3)]
    wt = []
    for (n, r, m), W in zip(GROUPS, Ws):
        wt.append(np.ascontiguousarray(W.reshape(m, n).T.astype(np.float32)))
    wt3 = np.ascontiguousarray(np.concatenate([wt[3], wt[3]], axis=0))  # (128, 256)

    cols = []
    b0 = bids[0]
    for kc in range(4):
        cols.append(b0[kc * 128 : (kc + 1) * 128])
    b1 = bids[1].reshape(256, 4)
    for kc in range(2):
        for rho in range(4):
            cols.append(b1[kc * 128 : (kc + 1) * 128, rho])
    b2 = bids[2].reshape(128, 16)
    for rho in range(16):
        cols.append(b2[:, rho])
    b3 = bids[3].reshape(64, 64)
    p = np.arange(128)
    for q in range(32):
        cols.append(b3[p % 64, 2 * q + p // 64])
    xgidx = np.ascontiguousarray(np.stack(cols, axis=1).astype(np.int32))  # (128, 60)

    ivp = np.asarray(W_invperm).reshape(256, 2, 128)  # [o, c, p]
    pidx = np.ascontiguousarray(
        ivp.transpose(2, 0, 1).reshape(128, 512).astype(np.int32)
    )

    onesld = np.zeros((128, 256), np.float32)
    onesld[:, 128] = 1.0

    return wt[0], wt[1], wt[2], wt3, xgidx, pidx, onesld


def _build_nc():
    nc = bacc.Bacc("TRN2", target_bir_lowering=False, debug=False, num_devices=NCORES)

    xs_d = nc.dram_tensor("xs", [BS, X], F32, kind="ExternalInput")
    wt_d = [
        nc.dram_tensor("wt0", [512, 16384], F32, kind="ExternalInput"),
        nc.dram_tensor("wt1", [256, 4096], F32, kind="ExternalInput"),
        nc.dram_tensor("wt2", [128, 1024], F32, kind="ExternalInput"),
        nc.dram_tensor("wt3", [128, 256], F32, kind="ExternalInput"),
    ]
    xgidx_d = nc.dram_tensor("xgidx", [128, 60], I32, kind="ExternalInput")
    pidx_d = nc.dram_tensor("pidx", [128, 512], I32, kind="ExternalInput")
    onesld_d = nc.dram_tensor("onesld", [128, 256], F32, kind="ExternalInput")
    out_d = nc.dram_tensor("out", [BS, X], F32, kind="ExternalOutput")

    with tile.TileContext(nc) as tc, ExitStack() as ctx:
        const = ctx.enter_context(tc.tile_pool(name="const", bufs=1))
        wpool = ctx.enter_context(tc.tile_pool(name="wpool", bufs=2))
        ypool = ctx.enter_context(tc.tile_pool(name="ypool", bufs=6))
        p2pool = ctx.enter_context(tc.tile_pool(name="p2pool", bufs=4))
        pgemm = ctx.enter_context(tc.tile_pool(name="pgemm", bufs=4, space="PSUM"))
        pacc = ctx.enter_context(tc.tile_pool(name="pacc", bufs=2, space="PSUM"))
        ptr = ctx.enter_context(tc.tile_pool(name="ptr", bufs=2, space="PSUM"))
        dram = ctx.enter_context(tc.tile_pool(name="dram", bufs=1, space="DRAM"))

        xT_dram = dram.tile([X, BS], F32)
        Y_dram = dram.tile([X * X, BS], F32)

        ident = const.tile([128, 128], F32)
        make_identity(nc, ident[:])

        xgidx_t = const.tile([128, 60], I32)
        nc.sync.dma_start(xgidx_t[:], xgidx_d[:])
        pidx_t = const.tile([128, 512], I32)
        nc.sync.dma_start(pidx_t[:], pidx_d[:])
        onesld_t = const.tile([128, 256], F32)
        nc.sync.dma_start(onesld_t[:], onesld_d[:])

        # ---- Phase 0: load x shard, transpose to xT, stage to DRAM ----
        xs0 = const.tile([128, X], F32)  # batch rows 0..127
        xs1 = const.tile([128, X], F32)  # batch rows 128..255
        nc.sync.dma_start(xs0[:], xs_d[0:128, :])
        nc.sync.dma_start(xs1[:], xs_d[128:256, :])

        xtp0 = const.tile([128, BS], F32)  # xT rows i=0..127 (plain)
        xtp1 = const.tile([128, BS], F32)  # xT rows i=128..255 (plain)
        xts = const.tile([128, 512], F32)  # [p, c*256+b] = 0.1*xT[c*128+p, b]
        for ih, xtp in ((0, xtp0), (1, xtp1)):
            for bh, xsrc in ((0, xs0), (1, xs1)):
                pst = ptr.tile([128, 128], F32)
                nc.tensor.transpose(
                    pst[:], xsrc[:, ih * 128 : (ih + 1) * 128], ident[:]
                )
                nc.vector.tensor_copy(xtp[:, bh * 128 : (bh + 1) * 128], pst[:])
                nc.vector.tensor_scalar_mul(
                    xts[:, ih * 256 + bh * 128 : ih * 256 + (bh + 1) * 128],
                    pst[:],
                    0.1,
                )
        nc.sync.dma_start(xT_dram[0:128, :], xtp0[:])
        nc.sync.dma_start(xT_dram[128:256, :], xtp1[:])

        # ---- Phase 1a: gather x_rep tiles from xT via indirect DMA ----
        # xrep0: 4 tiles (128, 256);   rhs chunk kc for g0
        # xrep1: 2 tiles (128, 1024);  [nu, rho*256+b] for g1
        # xrep2: 1 tile (128, 4096);   g2
        # xrep3: 1 tile (128, 8192);   [s*64+nu, q*256+b] = xT[bid3[nu*64+2q+s]]
        def igather(dst_slice, col):
            nc.gpsimd.indirect_dma_start(
                out=dst_slice,
                out_offset=None,
                in_=xT_dram[:],
                in_offset=IndirectOffsetOnAxis(ap=xgidx_t[:, col : col + 1], axis=0),
            )

        xrep0 = [const.tile([128, 256], F32, tag=f"xrep0_{kc}", name=f"xrep0_{kc}") for kc in range(4)]
        for kc in range(4):
            igather(xrep0[kc][:], kc)
        xrep1 = [const.tile([128, 1024], F32, tag=f"xrep1_{kc}", name=f"xrep1_{kc}") for kc in range(2)]
        for kc in range(2):
            for rho in range(4):
                igather(xrep1[kc][:, rho * 256 : (rho + 1) * 256], 4 + kc * 4 + rho)
        xrep2 = const.tile([128, 4096], F32)
        for rho in range(16):
            igather(xrep2[:, rho * 256 : (rho + 1) * 256], 12 + rho)
        xrep3 = const.tile([128, 8192], F32)
        for q in range(32):
            igather(xrep3[:, q * 256 : (q + 1) * 256], 28 + q)

        # ---- Phase 1b: group GEMMs -> Y ----
        # g0: Y rows 0..16383, k = mu
        for mp in range(16):
            w0t = [wpool.tile([128, 1024], F32, tag=f"w0_{kc}", name=f"w0_{kc}") for kc in range(4)]
            for kc in range(4):
                nc.sync.dma_start(
                    w0t[kc][:],
                    wt_d[0][kc * 128 : (kc + 1) * 128, mp * 1024 : (mp + 1) * 1024],
                )
            for mt in range(8):
                ps = pgemm.tile([128, 256], F32, tag="pg", name="ps")
                for kc in range(4):
                    nc.tensor.matmul(
                        ps[:],
                        w0t[kc][:, mt * 128 : (mt + 1) * 128],
                        xrep0[kc][:],
                        start=(kc == 0),
                        stop=(kc == 3),
                    )
                yt = ypool.tile([128, 256], F32, tag="ytile", name="yt")
                nc.any.tensor_copy(yt[:], ps[:])
                r0 = (mp * 8 + mt) * 128
                nc.sync.dma_start(Y_dram[r0 : r0 + 128, :], yt[:])

        # g1: Y rows 16384..32767, k = 16384 + mu*4 + rho
        w1t = [const.tile([128, 4096], F32, tag=f"w1_{kc}", name=f"w1_{kc}") for kc in range(2)]
        for kc in range(2):
            nc.sync.dma_start(w1t[kc][:], wt_d[1][kc * 128 : (kc + 1) * 128, :])
        Yr1 = Y_dram[:].rearrange("(a b) c -> a (b c)", b=4)  # (16384, 1024)
        for mt in range(32):
            for ns in range(2):
                ps = pgemm.tile([128, 512], F32, tag="pg", name="ps")
                for kc in range(2):
                    nc.tensor.matmul(
                        ps[:],
                        w1t[kc][:, mt * 128 : (mt + 1) * 128],
                        xrep1[kc][:, ns * 512 : (ns + 1) * 512],
                        start=(kc == 0),
                        stop=(kc == 1),
                    )
                yt = ypool.tile([128, 512], F32, tag="ytile", name="yt")
                nc.any.tensor_copy(yt[:], ps[:])
                nc.sync.dma_start(
                    Yr1[4096 + mt * 128 : 4096 + (mt + 1) * 128, ns * 512 : (ns + 1) * 512],
                    yt[:],
                )

        # g2: Y rows 32768..49151, k = 32768 + mu*16 + rho
        w2t = const.tile([128, 1024], F32)
        nc.sync.dma_start(w2t[:], wt_d[2][:])
        Yr2 = Y_dram[:].rearrange("(a b) c -> a (b c)", b=16)  # (4096, 4096)
        for mt in range(8):
            for ns in range(8):
                ps = pgemm.tile([128, 512], F32, tag="pg", name="ps")
                nc.tensor.matmul(
                    ps[:],
                    w2t[:, mt * 128 : (mt + 1) * 128],
                    xrep2[:, ns * 512 : (ns + 1) * 512],
                    start=True,
                    stop=True,
                )
                yt = ypool.tile([128, 512], F32, tag="ytile", name="yt")
                nc.any.tensor_copy(yt[:], ps[:])
                nc.sync.dma_start(
                    Yr2[2048 + mt * 128 : 2048 + (mt + 1) * 128, ns * 512 : (ns + 1) * 512],
                    yt[:],
                )

        # g3: Y rows 49152..65535, k = 49152 + mu*64 + 2q + s
        w3t = const.tile([128, 256], F32)
        nc.sync.dma_start(w3t[:], wt_d[3][:])
        Yr3 = Y_dram[49152:65536, :].rearrange(
            "(mu q s) c -> mu q (s c)", q=32, s=2
        )  # (256, 32, 512)
        for mt in range(2):
            for s in range(2):
                for ns in range(16):
                    ps = pgemm.tile([128, 512], F32, tag="pg", name="ps")
                    nc.tensor.matmul(
                        ps[:],
                        w3t[s * 64 : (s + 1) * 64, mt * 128 : (mt + 1) * 128],
                        xrep3[s * 64 : (s + 1) * 64, ns * 512 : (ns + 1) * 512],
                        start=True,
                        stop=True,
                    )
                    yt = ypool.tile([128, 512], F32, tag="ytile", name="yt")
                    nc.any.tensor_copy(yt[:], ps[:])
                    # rows mu = mt*128+p, q-pair = [2ns, 2ns+2), fixed s
                    nc.sync.dma_start(
                        Yr3[mt * 128 : (mt + 1) * 128, 2 * ns : 2 * ns + 2, s * 256 : (s + 1) * 256],
                        yt[:].rearrange("p (q c) -> p q c", c=256),
                    )

        # ---- Phase 2: permutation gather + contraction ----
        outstage = [const.tile([128, 256], F32, tag=f"outstage{bh}", name=f"outstage{bh}") for bh in range(2)]
        for ob in range(2):
            ps_acc = pacc.tile([128, 512], F32, tag="pacc", name="ps_acc")
            for j in range(128):
                o = ob * 128 + j
                yg = p2pool.tile([128, 512], F32, tag="ygath", name="yg")
                for c in range(2):
                    nc.gpsimd.indirect_dma_start(
                        out=yg[:, c * 256 : (c + 1) * 256],
                        out_offset=None,
                        in_=Y_dram[:],
                        in_offset=IndirectOffsetOnAxis(
                            ap=pidx_t[:, 2 * o + c : 2 * o + c + 1], axis=0
                        ),
                    )
                z = p2pool.tile([128, 512], F32, tag="z", name="z")
                nc.vector.tensor_mul(z[:], yg[:], xts[:])
                nc.tensor.matmul(
                    ps_acc[:],
                    onesld_t[:, 128 - j : 256 - j],
                    z[:],
                    start=(j == 0),
                    stop=(j == 127),
                )
            outT = p2pool.tile([128, 256], F32, tag="outT", name="outT")
            nc.vector.tensor_copy(outT[:], ps_acc[:, 0:256])
            nc.vector.tensor_add(outT[:], outT[:], ps_acc[:, 256:512])
            for bh in range(2):
                pst = ptr.tile([128, 128], F32)
                nc.tensor.transpose(
                    pst[:], outT[:, bh * 128 : (bh + 1) * 128], ident[:]
                )
                nc.any.tensor_copy(
                    outstage[bh][:, ob * 128 : (ob + 1) * 128], pst[:]
                )
        for bh in range(2):
            nc.sync.dma_start(out_d[bh * 128 : (bh + 1) * 128, :], outstage[bh][:])

    nc.compile()
    return nc


_NC_CACHE = None


def kernel(x, W0, W1, W2, W3, bid0, bid1, bid2, bid3, W_invperm, **_unused):
    global _NC_CACHE
    wt0, wt1, wt2, wt3, xgidx, pidx, onesld = _host_prep(
        W0, W1, W2, W3, bid0, bid1, bid2, bid3, W_invperm
    )
    if _NC_CACHE is None:
        _NC_CACHE = _build_nc()
    nc = _NC_CACHE

    x = np.ascontiguousarray(np.asarray(x, dtype=np.float32))
    in_maps = []
    for c in range(NCORES):
        in_maps.append(
            {
                "xs": x[c * BS : (c + 1) * BS, :],
                "wt0": wt0,
                "wt1": wt1,
                "wt2": wt2,
                "wt3": wt3,
                "xgidx": xgidx,
                "pidx": pidx,
                "onesld": onesld,
            }
        )
    res = run_bass_kernel_spmd(nc, in_maps, core_ids=list(range(NCORES)))
    out = np.concatenate([res.results[c]["out"] for c in range(NCORES)], axis=0)
    return out.astype(np.float32)
